# revision 41
# baseline (speedup 1.0000x reference)
"""Bass/Trainium2 kernel for nn_AttentionLayer_68229850464552.

Full multi-head causal attention layer (QKV proj + partial RoPE + attention +
output proj), head-sharded (tensor parallel) across 8 NeuronCores. Each core
computes 2 of the 16 heads for both batch elements and the partial output
projection for its heads' feature columns; the host scales by 1 and sums the
8 partials and adds the output bias.

Matmul operands are bf16 (PE streams 2B/lane/cycle -> 1 cycle/row); fp32
accumulation in PSUM throughout.

Self-contained: hardcodes shapes from the problem spec.
"""
import os
import numpy as np
import ml_dtypes
from contextlib import ExitStack

import concourse.bass as bass
import concourse.mybir as mybir
import concourse.tile as tile
from concourse import bacc
from concourse.bass_utils import run_bass_kernel_spmd

B, S, D, H, DK = 2, 2048, 2048, 16, 128
HPC = 2                      # heads per core
NCORES = 8
DR = 32                      # rope features
SCALE = 1.0 / float(np.sqrt(DK))
CH = 512                     # x seq-chunk width for the QKV projection
NCH = S // CH                # 4
QCW = 512                    # query chunk width in attention
NQC = S // QCW               # 4
NJ = S // 128                # 16 key blocks
WV_COLS = 2 * 128            # [v_h0 | v_h1]

F32 = mybir.dt.float32
F32R = mybir.dt.float32r
BF16 = mybir.dt.bfloat16
Act = mybir.ActivationFunctionType
Alu = mybir.AluOpType
BF_NP = ml_dtypes.bfloat16

_PROG_CACHE = {}


def _build_program():
    nc = bacc.Bacc("TRN2", target_bir_lowering=False, debug=False,
                   enable_asserts=True, num_devices=NCORES)

    # all weight/const tensors are partition-major on the host so DMAs are
    # contiguous per partition (fat descriptors)
    xpm = nc.dram_tensor("xpm", [128, B, NCH, 16, CH], BF16,
                         kind="ExternalInput").ap()
    wqa = nc.dram_tensor("wqa", [128, 16, 128], BF16, kind="ExternalInput").ap()
    wqb = nc.dram_tensor("wqb", [128, 16, 384], BF16, kind="ExternalInput").ap()
    wv = nc.dram_tensor("wv", [128, 16, WV_COLS], BF16,
                        kind="ExternalInput").ap()
    wo = nc.dram_tensor("wo", [128, HPC, D], BF16, kind="ExternalInput").ap()
    bqk = nc.dram_tensor("bqk", [128, 4], F32, kind="ExternalInput").ap()
    bv = nc.dram_tensor("bv", [128, WV_COLS], F32, kind="ExternalInput").ap()
    cos4 = nc.dram_tensor("cos4", [128, S], BF16, kind="ExternalInput").ap()
    sin4 = nc.dram_tensor("sin4", [128, S], BF16, kind="ExternalInput").ap()
    maskT = nc.dram_tensor("maskT", [128, 128], BF16, kind="ExternalInput").ap()
    idm = nc.dram_tensor("idm", [128, 128], BF16, kind="ExternalInput").ap()
    pout = nc.dram_tensor("pout", [B, 16, 128, D], BF16,
                          kind="ExternalOutput").ap()

    with tile.TileContext(nc) as tc, ExitStack() as ctx:
        wpool = ctx.enter_context(tc.tile_pool(name="w", bufs=1))
        xpool = ctx.enter_context(tc.tile_pool(name="x", bufs=3))
        qkpool = ctx.enter_context(tc.tile_pool(name="qk", bufs=2))
        vpool = ctx.enter_context(tc.tile_pool(name="v", bufs=2))
        otpool = ctx.enter_context(tc.tile_pool(name="ot", bufs=2))
        ppool = ctx.enter_context(tc.tile_pool(name="p", bufs=3))
        rpool = ctx.enter_context(tc.tile_pool(name="r", bufs=2))
        opool = ctx.enter_context(tc.tile_pool(name="o", bufs=3))
        ncpool = ctx.enter_context(tc.tile_pool(name="nc", bufs=3))
        prpool = ctx.enter_context(tc.tile_pool(name="pr", bufs=2))
        scpool = ctx.enter_context(tc.tile_pool(name="sc", bufs=3, space="PSUM"))
        accpool = ctx.enter_context(tc.tile_pool(name="acc", bufs=2,
                                                 space="PSUM"))
        pjpool = ctx.enter_context(tc.tile_pool(name="pj", bufs=3,
                                                space="PSUM"))

        # ---- startup: only what chunk 0/1 need, interleaved so PE starts
        # early; everything else is deferred until after the xt1 prefetch.
        wq0_sb = wpool.tile([128, 16, 128], BF16)
        xt0 = xpool.tile([128, 16, CH], BF16, tag="xt", name="xt0")
        nc.sync.dma_start(wq0_sb[:, 0:4, :], wqa[:, 0:4, :])
        nc.sync.dma_start(xt0[:, 0:2, :], xpm[:, 0, 0, 0:2, :])
        nc.sync.dma_start(wq0_sb[:, 4:16, :], wqa[:, 4:16, :])
        nc.sync.dma_start(xt0[:, 2:4, :], xpm[:, 0, 0, 2:4, :])
        nc.sync.dma_start(xt0[:, 4:8, :], xpm[:, 0, 0, 4:8, :])
        nc.sync.dma_start(xt0[:, 8:16, :], xpm[:, 0, 0, 8:16, :])
        bqk_sb = wpool.tile([128, 4], F32)
        nc.sync.dma_start(bqk_sb[:], bqk[:])
        wqb_sb = wpool.tile([128, 16, 384], BF16)
        nc.sync.dma_start(wqb_sb[:], wqb[:])
        wv_sb = wpool.tile([128, 16, WV_COLS], BF16)
        nc.sync.dma_start(wv_sb[:], wv[:])
        ones_sb = wpool.tile([128, 128], BF16)
        nc.gpsimd.memset(ones_sb[:], 1.0)
        # ramp the PE p-state to 2.4GHz while the startup DMAs stream: the
        # HAM only reaches full clock after ~3.4us of continuous PE activity
        junk_ps = pjpool.tile([128, 512], F32, tag="pj", name="junk")
        for _ in range(36):
            nc.tensor.matmul(junk_ps[:, 0:128], ones_sb[:], ones_sb[:],
                             start=True, stop=True)

        # declared here, loaded by deferred_consts() after xt1's prefetch is
        # in the DMA queue (they are only needed tens of us into the run)
        sin4_sb = wpool.tile([128, S], BF16)
        cos4_sb = wpool.tile([128, S], BF16)
        bv_sb = wpool.tile([128, WV_COLS], F32)
        maskT_sb = wpool.tile([128, 128], BF16)
        idm_sb = wpool.tile([128, 128], BF16)
        wo_sb = wpool.tile([128, HPC, D], BF16)

        def deferred_consts():
            nc.sync.dma_start(sin4_sb[:], sin4[:])
            nc.sync.dma_start(cos4_sb[:], cos4[:])
            nc.sync.dma_start(bv_sb[:], bv[:])
            nc.sync.dma_start(maskT_sb[:], maskT[:])
            nc.sync.dma_start(idm_sb[:], idm[:])
            nc.sync.dma_start(wo_sb[:], wo[:])

        def qkv_chunk(b, c, xt, qk_sb, v_sb, split_k=False):
            cs = slice(c * CH, (c + 1) * CH)

            def qk_mms(mt, ps, kts):
                for kt in kts:
                    w = (wq0_sb[:, kt, :] if mt == 0
                         else wqb_sb[:, kt, (mt - 1) * 128:mt * 128])
                    nc.tensor.matmul(ps[:], w, xt[:, kt, :],
                                     start=(kt == 0), stop=(kt == 15))

            if split_k:
                # chunk 0 only: run every group's first kt-half before any
                # second half so the PE isn't gated on the full xt stream
                pss = []
                for mt in range(3):
                    ps = pjpool.tile([128, CH], F32, tag="pj", name="ps")
                    qk_mms(mt, ps, range(8))
                    pss.append(ps)
                qk_mms(0, pss[0], range(8, 16))
                nc.scalar.activation(qk_sb[0][:, cs], pss[0][:], Act.Identity,
                                     bias=bqk_sb[:, 0:1])
                ps3 = pjpool.tile([128, CH], F32, tag="pj", name="ps")
                qk_mms(3, ps3, range(8))
                pss.append(ps3)
                for mt in range(1, 4):
                    qk_mms(mt, pss[mt], range(8, 16))
                    nc.scalar.activation(qk_sb[mt][:, cs], pss[mt][:],
                                         Act.Identity,
                                         bias=bqk_sb[:, mt:mt + 1])
            else:
                for mt in range(4):
                    ps = pjpool.tile([128, CH], F32, tag="pj", name="ps")
                    qk_mms(mt, ps, range(16))
                    nc.scalar.activation(qk_sb[mt][:, cs], ps[:], Act.Identity,
                                         bias=bqk_sb[:, mt:mt + 1])

            # RoPE, packed: all 4 tensors' rope rows on 128 partitions.
            # out = q·cos + shuf(q)·sin  (sin sign-folded on host)
            shuf = rpool.tile([128, CH], BF16, tag="shuf", name="shuf")
            ra = rpool.tile([128, CH], BF16, tag="ra", name="ra")
            for t in range(4):
                nc.sync.dma_start(shuf[32 * t:32 * t + 16, :],
                                  qk_sb[t][16:32, cs])
                nc.sync.dma_start(shuf[32 * t + 16:32 * t + 32, :],
                                  qk_sb[t][0:16, cs])
                nc.sync.dma_start(ra[32 * t:32 * t + 32, :],
                                  qk_sb[t][0:32, cs])
            tmp = rpool.tile([128, CH], F32, tag="rtmp", name="tmp")
            nc.vector.tensor_tensor(tmp[:], shuf[:], sin4_sb[:, cs], Alu.mult)
            ro = rpool.tile([128, CH], BF16, tag="ro", name="ro")
            nc.vector.tensor_tensor(ro[:], ra[:], cos4_sb[:, cs], Alu.mult)
            nc.vector.tensor_tensor(ro[:], ro[:], tmp[:], Alu.add)
            for t in range(4):
                nc.sync.dma_start(qk_sb[t][0:32, cs], ro[32 * t:32 * t + 32, :])

            # V projection for this chunk ([seq, feat] layout)
            for s2 in range(4):
                psv = pjpool.tile([128, WV_COLS], F32, tag="pj", name="psv")
                for kt in range(16):
                    nc.tensor.matmul(
                        psv[:], xt[:, kt, s2 * 128:(s2 + 1) * 128],
                        wv_sb[:, kt, :], start=(kt == 0), stop=(kt == 15))
                nc.vector.tensor_tensor(v_sb[:, c * 4 + s2, :], psv[:],
                                        bv_sb[:], Alu.add)

        def attn_unit(b, qc, h, qk_sb, v_sb, ot_sb, pe_sums=False, last=False):
            """Returns deferred norm closure.

            pe_sums: softmax denominators via per-j ones-matmuls on the PE
            (used where DVE/GpSimd partial accumulation doesn't work: qc==0
            has diag blocks as first touches, and the final unit can't absorb
            the partial-chain latency in its tail).  Otherwise exp tiles are
            accumulated elementwise on DVE/GpSimd (split by j parity) and
            reduced with two fp32r ones-matmuls at the end.
            """
            jmax = 4 * qc + 3
            otps = accpool.tile([128, QCW], F32, tag="acc", name="otps")
            sums = accpool.tile([128, QCW], F32, tag="acc", name="sums")
            if not pe_sums:
                part = [prpool.tile([128, QCW], BF16, tag=f"part{e}",
                                    name=f"part{e}") for e in range(2)]

            def emit_score(j):
                c0 = (j - 4 * qc) * 128 if j >= 4 * qc else 0
                diag = j >= 4 * qc
                sps = scpool.tile([128, QCW], F32, tag="sc", name="sps")
                nc.tensor.matmul(
                    sps[:, c0:QCW], qk_sb[2 + h][:, j * 128:(j + 1) * 128],
                    qk_sb[h][:, qc * QCW + c0:(qc + 1) * QCW],
                    start=True, stop=not diag)
                if diag:
                    # add -1e4 above the diagonal of the diag subblock
                    nc.tensor.matmul(
                        sps[:, c0:c0 + 128], maskT_sb[:], idm_sb[:],
                        start=False, stop=True)
                return sps

            def emit_consume(j, sps):
                c0 = (j - 4 * qc) * 128 if j >= 4 * qc else 0
                pt = ppool.tile([128, QCW], BF16, tag="pt", name="pt")
                nc.scalar.activation(pt[:, c0:QCW], sps[:, c0:QCW],
                                     Act.Exp, scale=SCALE)
                nc.tensor.matmul(
                    otps[:, c0:QCW],
                    v_sb[:, j, 128 * h:128 * (h + 1)],
                    pt[:, c0:QCW], start=(j == 0), stop=(j == jmax))
                if pe_sums:
                    nc.tensor.matmul(
                        sums[:, c0:QCW], ones_sb[:],
                        pt[:, c0:QCW], start=(j == 0), stop=(j == jmax))
                else:
                    eng = nc.vector if j % 2 == 0 else nc.gpsimd
                    tgt = part[j % 2]
                    if j < 2:
                        # j<2 are full blocks whenever qc>=1
                        nc.vector.tensor_copy(tgt[:], pt[:])
                    else:
                        with nc.allow_low_precision(
                                "bf16 softmax-denominator partials"):
                            eng.tensor_tensor(tgt[:, c0:QCW], tgt[:, c0:QCW],
                                              pt[:, c0:QCW], Alu.add)

            # 2-deep score lookahead: exp(j) gets ~2 blocks of PE work to
            # hide behind before pv(j) needs it (scpool holds 3 banks)
            sq = [emit_score(0), emit_score(1)]
            for j in range(2, jmax + 1):
                sq.append(emit_score(j))
                emit_consume(j - 2, sq.pop(0))
            emit_consume(jmax - 1, sq.pop(0))
            emit_consume(jmax, sq.pop(0))

            if not pe_sums:
                nc.tensor.matmul(sums[:], ones_sb[:], part[0][:],
                                 start=True, stop=False)
                nc.tensor.matmul(sums[:], ones_sb[:], part[1][:],
                                 start=False, stop=True)

            otr = ncpool.tile([128, QCW], F32, tag="otr", name="otr")
            rc = ncpool.tile([128, QCW], F32, tag="rc", name="rc")
            if last:
                # slice the norm so the final outproj can start per-sblk
                def norm():
                    for s in range(4):
                        sl = slice(s * 128, (s + 1) * 128)
                        nc.scalar.activation(otr[:, sl], otps[:, sl], Act.Copy)
                        nc.vector.reciprocal_approx_fast(rc[:, sl],
                                                         sums[:, sl])
                        nc.gpsimd.tensor_tensor(
                            ot_sb[:, h, qc * QCW + s * 128:
                                  qc * QCW + (s + 1) * 128],
                            otr[:, sl], rc[:, sl], Alu.mult)
                return norm
            nc.vector.tensor_copy(otr[:], otps[:])
            nc.vector.reciprocal_approx_fast(rc[:], sums[:])

            def norm():
                nc.gpsimd.tensor_tensor(ot_sb[:, h, qc * QCW:(qc + 1) * QCW],
                                        otr[:], rc[:], Alu.mult)
            return norm

        def outproj(b, qc, ot_sb):
            for sblk in range(4 * qc, 4 * qc + 4):
                po = opool.tile([128, D], BF16, tag="po", name="po")
                for n in range(4):
                    psc = pjpool.tile([128, 512], F32, tag="pj", name="psc")
                    for kt in range(HPC):
                        nc.tensor.matmul(
                            psc[:], ot_sb[:, kt, sblk * 128:(sblk + 1) * 128],
                            wo_sb[:, kt, n * 512:(n + 1) * 512],
                            start=(kt == 0), stop=(kt == 1))
                    if n % 2 == 0:
                        nc.vector.tensor_copy(po[:, n * 512:(n + 1) * 512],
                                              psc[:])
                    else:
                        nc.scalar.activation(po[:, n * 512:(n + 1) * 512],
                                             psc[:], Act.Copy)
                    if n % 2 == 1:
                        nc.sync.dma_start(
                            pout[b, sblk, :, (n - 1) * 512:(n + 1) * 512],
                            po[:, (n - 1) * 512:(n + 1) * 512])

        # ---------------- batch 0 QKV ----------------
        qk0 = [qkpool.tile([128, S], BF16, tag=f"qk{t}", name=f"qk{t}_b0")
               for t in range(4)]
        v0 = vpool.tile([128, NJ, WV_COLS], BF16, tag="v", name="v_b0")
        xt_cur = xt0
        for c in range(NCH):
            if c + 1 < NCH:
                xt_next = xpool.tile([128, 16, CH], BF16, tag="xt", name="xtn")
                nc.sync.dma_start(xt_next[:], xpm[:, 0, c + 1, :, :])
            else:
                xt_next = xpool.tile([128, 16, CH], BF16, tag="xt", name="xtn")
                nc.sync.dma_start(xt_next[:], xpm[:, 1, 0, :, :])
            if c == 0:
                deferred_consts()
            qkv_chunk(0, c, xt_cur, qk0, v0)
            xt_cur = xt_next

        # ---- attention: b0 units, b1 QKV, and b1 units interleaved ----
        qk1 = [qkpool.tile([128, S], BF16, tag=f"qk{t}", name=f"qk{t}_b1")
               for t in range(4)]
        v1 = vpool.tile([128, NJ, WV_COLS], BF16, tag="v", name="v_b1")
        ot0 = otpool.tile([128, HPC, S], BF16, tag="ot", name="ot_b0")
        ot1 = otpool.tile([128, HPC, S], BF16, tag="ot", name="ot_b1")
        norm_pending = None

        def unit(b, qc, h, pe_sums=False, last=False):
            nonlocal norm_pending
            nrm = attn_unit(b, qc, h, qk1 if b else qk0, v1 if b else v0,
                            ot1 if b else ot0, pe_sums=pe_sums, last=last)
            if norm_pending is not None:
                norm_pending()
            norm_pending = nrm

        for qc in range(NQC):
            # batch 1 chunk qc QKV first: its ACT identities land ahead of
            # the units' exps in the in-order ACT queue, so a stalled
            # identity can't block scores that become ready later
            if qc + 1 < NCH:
                xt_next = xpool.tile([128, 16, CH], BF16, tag="xt", name="xtn")
                nc.sync.dma_start(xt_next[:], xpm[:, 1, qc + 1, :, :])
            qkv_chunk(1, qc, xt_cur, qk1, v1)
            xt_cur = xt_next if qc + 1 < NCH else None
            for h in range(HPC):
                unit(0, qc, h, pe_sums=(qc == 0))
            if qc >= 1:
                for h in range(HPC):
                    unit(1, qc - 1, h, pe_sums=(qc - 1 == 0))
            if qc == 1:
                outproj(0, 0, ot0)
            if qc >= 2:
                outproj(0, qc - 1, ot0)
                outproj(1, qc - 2, ot1)
        outproj(0, 3, ot0)
        unit(1, 3, 0)
        outproj(1, 2, ot1)
        unit(1, 3, 1, pe_sums=True, last=True)
        norm_pending()
        outproj(1, 3, ot1)

    nc.compile()
    return nc


def kernel(x, W_qkv, b_qkv, W_out, b_out):
    x = np.asarray(x, dtype=np.float32)
    W_qkv = np.asarray(W_qkv, dtype=np.float32)
    b_qkv = np.asarray(b_qkv, dtype=np.float32)
    W_out = np.asarray(W_out, dtype=np.float32)
    b_out = np.asarray(b_out, dtype=np.float32)

    if "prog" not in _PROG_CACHE:
        _PROG_CACHE["prog"] = _build_program()
    nc = _PROG_CACHE["prog"]

    # x -> [p, b, c, kt, s'] partition-major layout
    xpm = np.ascontiguousarray(
        x.reshape(B, NCH, CH, 16, 128).transpose(4, 0, 1, 3, 2)
    ).astype(BF_NP)

    i = np.arange(16, dtype=np.float64)
    theta = 1.0 / (10000.0 ** ((2.0 * i) / DR))
    s_idx = np.arange(S, dtype=np.float64)
    idx = s_idx[:, None] * theta[None, :]          # [S, 16]
    idx2 = np.concatenate([idx, idx], axis=1)      # [S, 32]
    cosT = np.cos(idx2).T.astype(np.float32)       # [32, S]
    sinT = np.sin(idx2).T.astype(np.float32)
    sinT[0:16, :] *= -1.0      # sign of rot = [-q[16:32], q[0:16]] folded in
    cos4 = np.ascontiguousarray(np.tile(cosT, (4, 1)).astype(BF_NP))  # [128,S]
    sin4 = np.ascontiguousarray(np.tile(sinT, (4, 1)).astype(BF_NP))

    maskT = np.triu(np.full((128, 128), -10000.0, dtype=np.float32),
                    1).astype(BF_NP)
    idm = np.eye(128, dtype=np.float32).astype(BF_NP)

    def part_major(w_cols):
        # w_cols: [D, M] -> [128, D//128, M] partition-major
        M = w_cols.shape[1]
        return np.ascontiguousarray(
            w_cols.reshape(16, 128, M).transpose(1, 0, 2))

    in_maps = []
    for c in range(NCORES):
        heads = [HPC * c, HPC * c + 1]
        qw, kw, vw, qb, kb, vb = [], [], [], [], [], []
        for hh in heads:
            base = 3 * DK * hh
            qw.append(W_qkv[base:base + 128])
            kw.append(W_qkv[base + 128:base + 256])
            vw.append(W_qkv[base + 256:base + 384])
            qb.append(b_qkv[base:base + 128])
            kb.append(b_qkv[base + 128:base + 256])
            vb.append(b_qkv[base + 256:base + 384])

        wq_full = np.concatenate([qw[0], qw[1], kw[0], kw[1]], axis=0).T
        wq_pm = part_major(wq_full).astype(BF_NP)        # [128, 16, 512]
        wqa = np.ascontiguousarray(wq_pm[:, :, 0:128])
        wqb = np.ascontiguousarray(wq_pm[:, :, 128:512])

        wv_full = np.concatenate([vw[0], vw[1]], axis=0).T   # [D, 256]
        wv_pm = np.ascontiguousarray(part_major(wv_full).astype(BF_NP))

        bv_np = np.concatenate([vb[0], vb[1]])[None, :]
        bv_np = np.ascontiguousarray(np.repeat(bv_np, 128, axis=0))

        bqk_np = np.zeros((128, 4), dtype=np.float32)
        bqk_np[:, 0] = qb[0]
        bqk_np[:, 1] = qb[1]
        bqk_np[:, 2] = kb[0]
        bqk_np[:, 3] = kb[1]

        wo_full = W_out[:, HPC * DK * c: HPC * DK * (c + 1)].T   # [256, D]
        wo_pm = np.ascontiguousarray(
            wo_full.reshape(2, 128, D).transpose(1, 0, 2)).astype(BF_NP)

        in_maps.append({
            "xpm": xpm, "wqa": wqa, "wqb": wqb, "wv": wv_pm, "wo": wo_pm,
            "bqk": bqk_np, "bv": bv_np, "cos4": cos4, "sin4": sin4,
            "maskT": maskT, "idm": idm,
        })

    trace = os.environ.get("KERNEL_TRACE", "0") == "1"
    res = run_bass_kernel_spmd(nc, in_maps, core_ids=list(range(NCORES)),
                               trace=trace)
    if res.exec_time_ns is not None:
        print(f"HW exec time: {res.exec_time_ns} ns")
        if res.instructions_and_trace is not None:
            print(f"trace: {res.instructions_and_trace[1]}")

    acc = np.zeros((B * S, D), dtype=np.float32)
    for c in range(NCORES):
        acc += res.results[c]["pout"].astype(np.float32).reshape(B * S, D)
    out = acc + b_out[None, :]
    return out.reshape(B, S, D)


# revision 42
# speedup vs baseline: 1.2024x; 1.2024x over previous
"""Bass/Trainium2 kernel for nn_AttentionLayer_68229850464552.

Full multi-head causal attention layer (QKV proj + partial RoPE + attention +
output proj), head-sharded (tensor parallel) across 8 NeuronCores. Each core
computes 2 of the 16 heads for both batch elements and the partial output
projection for its heads' feature columns; the host scales by 1 and sums the
8 partials and adds the output bias.

Matmul operands are bf16 (PE streams 2B/lane/cycle -> 1 cycle/row); fp32
accumulation in PSUM throughout.

Self-contained: hardcodes shapes from the problem spec.
"""
import os
import numpy as np
import ml_dtypes
from contextlib import ExitStack

import concourse.bass as bass
import concourse.mybir as mybir
import concourse.tile as tile
from concourse import bacc
from concourse.bass_utils import run_bass_kernel_spmd

B, S, D, H, DK = 2, 2048, 2048, 16, 128
HPC = 2                      # heads per core
NCORES = 8
DR = 32                      # rope features
SCALE = 1.0 / float(np.sqrt(DK))
CH = 512                     # x seq-chunk width for the QKV projection
NCH = S // CH                # 4
QCW = 512                    # query chunk width in attention
NQC = S // QCW               # 4
NJ = S // 128                # 16 key blocks
WV_COLS = 2 * 128            # [v_h0 | v_h1]

F32 = mybir.dt.float32
F32R = mybir.dt.float32r
BF16 = mybir.dt.bfloat16
Act = mybir.ActivationFunctionType
Alu = mybir.AluOpType
BF_NP = ml_dtypes.bfloat16

_PROG_CACHE = {}


def _build_program():
    nc = bacc.Bacc("TRN2", target_bir_lowering=False, debug=False,
                   enable_asserts=True, num_devices=NCORES)

    # all weight/const tensors are partition-major on the host so DMAs are
    # contiguous per partition (fat descriptors)
    xpm = nc.dram_tensor("xpm", [128, B, NCH, 16, CH], BF16,
                         kind="ExternalInput").ap()
    wqa = nc.dram_tensor("wqa", [128, 16, 128], BF16, kind="ExternalInput").ap()
    wqb = nc.dram_tensor("wqb", [128, 16, 384], BF16, kind="ExternalInput").ap()
    wv = nc.dram_tensor("wv", [128, 16, WV_COLS], BF16,
                        kind="ExternalInput").ap()
    wo = nc.dram_tensor("wo", [128, HPC, D], BF16, kind="ExternalInput").ap()
    bqk = nc.dram_tensor("bqk", [128, 4], F32, kind="ExternalInput").ap()
    bv = nc.dram_tensor("bv", [128, WV_COLS], F32, kind="ExternalInput").ap()
    cos4 = nc.dram_tensor("cos4", [128, S], BF16, kind="ExternalInput").ap()
    sin4 = nc.dram_tensor("sin4", [128, S], BF16, kind="ExternalInput").ap()
    maskT = nc.dram_tensor("maskT", [128, 128], BF16, kind="ExternalInput").ap()
    idm = nc.dram_tensor("idm", [128, 128], BF16, kind="ExternalInput").ap()
    pout = nc.dram_tensor("pout", [B, 16, 128, D], BF16,
                          kind="ExternalOutput").ap()

    with tile.TileContext(nc) as tc, ExitStack() as ctx:
        wpool = ctx.enter_context(tc.tile_pool(name="w", bufs=1))
        xpool = ctx.enter_context(tc.tile_pool(name="x", bufs=3))
        qkpool = ctx.enter_context(tc.tile_pool(name="qk", bufs=2))
        vpool = ctx.enter_context(tc.tile_pool(name="v", bufs=2))
        otpool = ctx.enter_context(tc.tile_pool(name="ot", bufs=2))
        ppool = ctx.enter_context(tc.tile_pool(name="p", bufs=6))
        rpool = ctx.enter_context(tc.tile_pool(name="r", bufs=2))
        opool = ctx.enter_context(tc.tile_pool(name="o", bufs=3))
        ncpool = ctx.enter_context(tc.tile_pool(name="nc", bufs=3))
        prpool = ctx.enter_context(tc.tile_pool(name="pr", bufs=2))
        scpool = ctx.enter_context(tc.tile_pool(name="sc", bufs=3, space="PSUM"))
        accpool = ctx.enter_context(tc.tile_pool(name="acc", bufs=2,
                                                 space="PSUM"))
        pjpool = ctx.enter_context(tc.tile_pool(name="pj", bufs=3,
                                                space="PSUM"))

        # ---- startup: only what chunk 0/1 need, interleaved so PE starts
        # early; everything else is deferred until after the xt1 prefetch.
        wq0_sb = wpool.tile([128, 16, 128], BF16)
        xt0 = xpool.tile([128, 16, CH], BF16, tag="xt", name="xt0")
        nc.sync.dma_start(wq0_sb[:, 0:4, :], wqa[:, 0:4, :])
        nc.sync.dma_start(xt0[:, 0:2, :], xpm[:, 0, 0, 0:2, :])
        nc.sync.dma_start(wq0_sb[:, 4:16, :], wqa[:, 4:16, :])
        nc.sync.dma_start(xt0[:, 2:4, :], xpm[:, 0, 0, 2:4, :])
        nc.sync.dma_start(xt0[:, 4:8, :], xpm[:, 0, 0, 4:8, :])
        nc.sync.dma_start(xt0[:, 8:16, :], xpm[:, 0, 0, 8:16, :])
        bqk_sb = wpool.tile([128, 4], F32)
        nc.sync.dma_start(bqk_sb[:], bqk[:])
        wqb_sb = wpool.tile([128, 16, 384], BF16)
        nc.sync.dma_start(wqb_sb[:], wqb[:])
        wv_sb = wpool.tile([128, 16, WV_COLS], BF16)
        nc.sync.dma_start(wv_sb[:], wv[:])
        ones_sb = wpool.tile([128, 128], BF16)
        nc.gpsimd.memset(ones_sb[:], 1.0)
        # ramp the PE p-state to 2.4GHz while the startup DMAs stream: the
        # HAM only reaches full clock after ~3.4us of continuous PE activity
        junk_ps = pjpool.tile([128, 512], F32, tag="pj", name="junk")
        for _ in range(36):
            nc.tensor.matmul(junk_ps[:, 0:128], ones_sb[:], ones_sb[:],
                             start=True, stop=True)

        # declared here, loaded by deferred_consts() after xt1's prefetch is
        # in the DMA queue (they are only needed tens of us into the run)
        sin4_sb = wpool.tile([128, S], BF16)
        cos4_sb = wpool.tile([128, S], BF16)
        bv_sb = wpool.tile([128, WV_COLS], F32)
        maskT_sb = wpool.tile([128, 128], BF16)
        idm_sb = wpool.tile([128, 128], BF16)
        wo_sb = wpool.tile([128, HPC, D], BF16)

        def deferred_consts():
            nc.sync.dma_start(sin4_sb[:], sin4[:])
            nc.sync.dma_start(cos4_sb[:], cos4[:])
            nc.sync.dma_start(bv_sb[:], bv[:])
            nc.sync.dma_start(maskT_sb[:], maskT[:])
            nc.sync.dma_start(idm_sb[:], idm[:])
            nc.sync.dma_start(wo_sb[:], wo[:])

        def qkv_chunk(b, c, xt, qk_sb, v_sb, split_k=False):
            cs = slice(c * CH, (c + 1) * CH)

            def qk_mms(mt, ps, kts):
                for kt in kts:
                    w = (wq0_sb[:, kt, :] if mt == 0
                         else wqb_sb[:, kt, (mt - 1) * 128:mt * 128])
                    nc.tensor.matmul(ps[:], w, xt[:, kt, :],
                                     start=(kt == 0), stop=(kt == 15))

            if split_k:
                # chunk 0 only: run every group's first kt-half before any
                # second half so the PE isn't gated on the full xt stream
                pss = []
                for mt in range(3):
                    ps = pjpool.tile([128, CH], F32, tag="pj", name="ps")
                    qk_mms(mt, ps, range(8))
                    pss.append(ps)
                qk_mms(0, pss[0], range(8, 16))
                nc.scalar.activation(qk_sb[0][:, cs], pss[0][:], Act.Identity,
                                     bias=bqk_sb[:, 0:1])
                ps3 = pjpool.tile([128, CH], F32, tag="pj", name="ps")
                qk_mms(3, ps3, range(8))
                pss.append(ps3)
                for mt in range(1, 4):
                    qk_mms(mt, pss[mt], range(8, 16))
                    nc.scalar.activation(qk_sb[mt][:, cs], pss[mt][:],
                                         Act.Identity,
                                         bias=bqk_sb[:, mt:mt + 1])
            else:
                for mt in range(4):
                    ps = pjpool.tile([128, CH], F32, tag="pj", name="ps")
                    qk_mms(mt, ps, range(16))
                    nc.scalar.activation(qk_sb[mt][:, cs], ps[:], Act.Identity,
                                         bias=bqk_sb[:, mt:mt + 1])

            # RoPE, packed: all 4 tensors' rope rows on 128 partitions.
            # out = q·cos + shuf(q)·sin  (sin sign-folded on host)
            shuf = rpool.tile([128, CH], BF16, tag="shuf", name="shuf")
            ra = rpool.tile([128, CH], BF16, tag="ra", name="ra")
            for t in range(4):
                nc.sync.dma_start(shuf[32 * t:32 * t + 16, :],
                                  qk_sb[t][16:32, cs])
                nc.sync.dma_start(shuf[32 * t + 16:32 * t + 32, :],
                                  qk_sb[t][0:16, cs])
                nc.sync.dma_start(ra[32 * t:32 * t + 32, :],
                                  qk_sb[t][0:32, cs])
            tmp = rpool.tile([128, CH], F32, tag="rtmp", name="tmp")
            nc.vector.tensor_tensor(tmp[:], shuf[:], sin4_sb[:, cs], Alu.mult)
            ro = rpool.tile([128, CH], BF16, tag="ro", name="ro")
            nc.vector.tensor_tensor(ro[:], ra[:], cos4_sb[:, cs], Alu.mult)
            nc.vector.tensor_tensor(ro[:], ro[:], tmp[:], Alu.add)
            for t in range(4):
                nc.sync.dma_start(qk_sb[t][0:32, cs], ro[32 * t:32 * t + 32, :])

            # V projection for this chunk ([seq, feat] layout)
            for s2 in range(4):
                psv = pjpool.tile([128, WV_COLS], F32, tag="pj", name="psv")
                for kt in range(16):
                    nc.tensor.matmul(
                        psv[:], xt[:, kt, s2 * 128:(s2 + 1) * 128],
                        wv_sb[:, kt, :], start=(kt == 0), stop=(kt == 15))
                nc.vector.tensor_tensor(v_sb[:, c * 4 + s2, :], psv[:],
                                        bv_sb[:], Alu.add)

        def attn_unit(b, qc, h, qk_sb, v_sb, ot_sb, pe_sums=False, last=False):
            """Returns deferred norm closure.

            pe_sums: softmax denominators via per-j ones-matmuls on the PE
            (used where DVE/GpSimd partial accumulation doesn't work: qc==0
            has diag blocks as first touches, and the final unit can't absorb
            the partial-chain latency in its tail).  Otherwise exp tiles are
            accumulated elementwise on DVE/GpSimd (split by j parity) and
            reduced with two fp32r ones-matmuls at the end.
            """
            jmax = 4 * qc + 3
            otps = accpool.tile([128, QCW], F32, tag="acc", name="otps")
            sums = accpool.tile([128, QCW], F32, tag="acc", name="sums")
            if not pe_sums:
                part = [prpool.tile([128, QCW], BF16, tag=f"part{e}",
                                    name=f"part{e}") for e in range(2)]

            def emit_score(j):
                c0 = (j - 4 * qc) * 128 if j >= 4 * qc else 0
                diag = j >= 4 * qc
                sps = scpool.tile([128, QCW], F32, tag="sc", name="sps")
                nc.tensor.matmul(
                    sps[:, c0:QCW], qk_sb[2 + h][:, j * 128:(j + 1) * 128],
                    qk_sb[h][:, qc * QCW + c0:(qc + 1) * QCW],
                    start=True, stop=not diag)
                if diag:
                    # add -1e4 above the diagonal of the diag subblock
                    nc.tensor.matmul(
                        sps[:, c0:c0 + 128], maskT_sb[:], idm_sb[:],
                        start=False, stop=True)
                return sps

            def emit_consume(j, sps):
                c0 = (j - 4 * qc) * 128 if j >= 4 * qc else 0
                pt = ppool.tile([128, QCW], BF16, tag="pt", name="pt")
                nc.scalar.activation(pt[:, c0:QCW], sps[:, c0:QCW],
                                     Act.Exp, scale=SCALE)
                nc.tensor.matmul(
                    otps[:, c0:QCW],
                    v_sb[:, j, 128 * h:128 * (h + 1)],
                    pt[:, c0:QCW], start=(j == 0), stop=(j == jmax))
                if pe_sums:
                    nc.tensor.matmul(
                        sums[:, c0:QCW], ones_sb[:],
                        pt[:, c0:QCW], start=(j == 0), stop=(j == jmax))
                else:
                    eng = nc.vector if j % 2 == 0 else nc.gpsimd
                    tgt = part[j % 2]
                    if j < 2:
                        # j<2 are full blocks whenever qc>=1
                        nc.vector.tensor_copy(tgt[:], pt[:])
                    else:
                        with nc.allow_low_precision(
                                "bf16 softmax-denominator partials"):
                            eng.tensor_tensor(tgt[:, c0:QCW], tgt[:, c0:QCW],
                                              pt[:, c0:QCW], Alu.add)

            # 2-deep score lookahead: exp(j) gets ~2 blocks of PE work to
            # hide behind before pv(j) needs it (scpool holds 3 banks)
            sq = [emit_score(0), emit_score(1)]
            for j in range(2, jmax + 1):
                sq.append(emit_score(j))
                emit_consume(j - 2, sq.pop(0))
            emit_consume(jmax - 1, sq.pop(0))
            emit_consume(jmax, sq.pop(0))

            if not pe_sums:
                nc.tensor.matmul(sums[:], ones_sb[:], part[0][:],
                                 start=True, stop=False)
                nc.tensor.matmul(sums[:], ones_sb[:], part[1][:],
                                 start=False, stop=True)

            otr = ncpool.tile([128, QCW], F32, tag="otr", name="otr")
            rc = ncpool.tile([128, QCW], F32, tag="rc", name="rc")
            if last:
                # slice the norm so the final outproj can start per-sblk
                def norm():
                    for s in range(4):
                        sl = slice(s * 128, (s + 1) * 128)
                        nc.scalar.activation(otr[:, sl], otps[:, sl], Act.Copy)
                        nc.vector.reciprocal_approx_fast(rc[:, sl],
                                                         sums[:, sl])
                        nc.gpsimd.tensor_tensor(
                            ot_sb[:, h, qc * QCW + s * 128:
                                  qc * QCW + (s + 1) * 128],
                            otr[:, sl], rc[:, sl], Alu.mult)
                return norm
            nc.vector.tensor_copy(otr[:], otps[:])
            nc.vector.reciprocal_approx_fast(rc[:], sums[:])

            def norm():
                nc.gpsimd.tensor_tensor(ot_sb[:, h, qc * QCW:(qc + 1) * QCW],
                                        otr[:], rc[:], Alu.mult)
            return norm

        def outproj(b, qc, ot_sb):
            for sblk in range(4 * qc, 4 * qc + 4):
                po = opool.tile([128, D], BF16, tag="po", name="po")
                for n in range(4):
                    psc = pjpool.tile([128, 512], F32, tag="pj", name="psc")
                    for kt in range(HPC):
                        nc.tensor.matmul(
                            psc[:], ot_sb[:, kt, sblk * 128:(sblk + 1) * 128],
                            wo_sb[:, kt, n * 512:(n + 1) * 512],
                            start=(kt == 0), stop=(kt == 1))
                    if n % 2 == 0:
                        nc.vector.tensor_copy(po[:, n * 512:(n + 1) * 512],
                                              psc[:])
                    else:
                        nc.scalar.activation(po[:, n * 512:(n + 1) * 512],
                                             psc[:], Act.Copy)
                    if n % 2 == 1:
                        nc.sync.dma_start(
                            pout[b, sblk, :, (n - 1) * 512:(n + 1) * 512],
                            po[:, (n - 1) * 512:(n + 1) * 512])

        # ---------------- batch 0 QKV ----------------
        qk0 = [qkpool.tile([128, S], BF16, tag=f"qk{t}", name=f"qk{t}_b0")
               for t in range(4)]
        v0 = vpool.tile([128, NJ, WV_COLS], BF16, tag="v", name="v_b0")
        xt_cur = xt0
        for c in range(NCH):
            if c + 1 < NCH:
                xt_next = xpool.tile([128, 16, CH], BF16, tag="xt", name="xtn")
                nc.sync.dma_start(xt_next[:], xpm[:, 0, c + 1, :, :])
            else:
                xt_next = xpool.tile([128, 16, CH], BF16, tag="xt", name="xtn")
                nc.sync.dma_start(xt_next[:], xpm[:, 1, 0, :, :])
            if c == 0:
                deferred_consts()
            qkv_chunk(0, c, xt_cur, qk0, v0)
            xt_cur = xt_next

        # ---- attention: b0 units, b1 QKV, and b1 units interleaved ----
        qk1 = [qkpool.tile([128, S], BF16, tag=f"qk{t}", name=f"qk{t}_b1")
               for t in range(4)]
        v1 = vpool.tile([128, NJ, WV_COLS], BF16, tag="v", name="v_b1")
        ot0 = otpool.tile([128, HPC, S], BF16, tag="ot", name="ot_b0")
        ot1 = otpool.tile([128, HPC, S], BF16, tag="ot", name="ot_b1")
        norm_pending = None

        def unit(b, qc, h, pe_sums=False, last=False):
            nonlocal norm_pending
            nrm = attn_unit(b, qc, h, qk1 if b else qk0, v1 if b else v0,
                            ot1 if b else ot0, pe_sums=pe_sums, last=last)
            if norm_pending is not None:
                norm_pending()
            norm_pending = nrm

        for qc in range(NQC):
            for h in range(HPC):
                unit(0, qc, h, pe_sums=(qc == 0))
            # batch 1 chunk qc QKV goes here to fill PE bubbles
            if qc + 1 < NCH:
                xt_next = xpool.tile([128, 16, CH], BF16, tag="xt", name="xtn")
                nc.sync.dma_start(xt_next[:], xpm[:, 1, qc + 1, :, :])
            qkv_chunk(1, qc, xt_cur, qk1, v1)
            xt_cur = xt_next if qc + 1 < NCH else None
            if qc >= 1:
                for h in range(HPC):
                    unit(1, qc - 1, h, pe_sums=(qc - 1 == 0))
            if qc == 1:
                outproj(0, 0, ot0)
            if qc >= 2:
                outproj(0, qc - 1, ot0)
                outproj(1, qc - 2, ot1)
        outproj(0, 3, ot0)
        unit(1, 3, 0)
        outproj(1, 2, ot1)
        unit(1, 3, 1, pe_sums=True, last=True)
        norm_pending()
        outproj(1, 3, ot1)

    nc.compile()
    return nc


def kernel(x, W_qkv, b_qkv, W_out, b_out):
    x = np.asarray(x, dtype=np.float32)
    W_qkv = np.asarray(W_qkv, dtype=np.float32)
    b_qkv = np.asarray(b_qkv, dtype=np.float32)
    W_out = np.asarray(W_out, dtype=np.float32)
    b_out = np.asarray(b_out, dtype=np.float32)

    if "prog" not in _PROG_CACHE:
        _PROG_CACHE["prog"] = _build_program()
    nc = _PROG_CACHE["prog"]

    # x -> [p, b, c, kt, s'] partition-major layout
    xpm = np.ascontiguousarray(
        x.reshape(B, NCH, CH, 16, 128).transpose(4, 0, 1, 3, 2)
    ).astype(BF_NP)

    i = np.arange(16, dtype=np.float64)
    theta = 1.0 / (10000.0 ** ((2.0 * i) / DR))
    s_idx = np.arange(S, dtype=np.float64)
    idx = s_idx[:, None] * theta[None, :]          # [S, 16]
    idx2 = np.concatenate([idx, idx], axis=1)      # [S, 32]
    cosT = np.cos(idx2).T.astype(np.float32)       # [32, S]
    sinT = np.sin(idx2).T.astype(np.float32)
    sinT[0:16, :] *= -1.0      # sign of rot = [-q[16:32], q[0:16]] folded in
    cos4 = np.ascontiguousarray(np.tile(cosT, (4, 1)).astype(BF_NP))  # [128,S]
    sin4 = np.ascontiguousarray(np.tile(sinT, (4, 1)).astype(BF_NP))

    maskT = np.triu(np.full((128, 128), -10000.0, dtype=np.float32),
                    1).astype(BF_NP)
    idm = np.eye(128, dtype=np.float32).astype(BF_NP)

    def part_major(w_cols):
        # w_cols: [D, M] -> [128, D//128, M] partition-major
        M = w_cols.shape[1]
        return np.ascontiguousarray(
            w_cols.reshape(16, 128, M).transpose(1, 0, 2))

    in_maps = []
    for c in range(NCORES):
        heads = [HPC * c, HPC * c + 1]
        qw, kw, vw, qb, kb, vb = [], [], [], [], [], []
        for hh in heads:
            base = 3 * DK * hh
            qw.append(W_qkv[base:base + 128])
            kw.append(W_qkv[base + 128:base + 256])
            vw.append(W_qkv[base + 256:base + 384])
            qb.append(b_qkv[base:base + 128])
            kb.append(b_qkv[base + 128:base + 256])
            vb.append(b_qkv[base + 256:base + 384])

        wq_full = np.concatenate([qw[0], qw[1], kw[0], kw[1]], axis=0).T
        wq_pm = part_major(wq_full).astype(BF_NP)        # [128, 16, 512]
        wqa = np.ascontiguousarray(wq_pm[:, :, 0:128])
        wqb = np.ascontiguousarray(wq_pm[:, :, 128:512])

        wv_full = np.concatenate([vw[0], vw[1]], axis=0).T   # [D, 256]
        wv_pm = np.ascontiguousarray(part_major(wv_full).astype(BF_NP))

        bv_np = np.concatenate([vb[0], vb[1]])[None, :]
        bv_np = np.ascontiguousarray(np.repeat(bv_np, 128, axis=0))

        bqk_np = np.zeros((128, 4), dtype=np.float32)
        bqk_np[:, 0] = qb[0]
        bqk_np[:, 1] = qb[1]
        bqk_np[:, 2] = kb[0]
        bqk_np[:, 3] = kb[1]

        wo_full = W_out[:, HPC * DK * c: HPC * DK * (c + 1)].T   # [256, D]
        wo_pm = np.ascontiguousarray(
            wo_full.reshape(2, 128, D).transpose(1, 0, 2)).astype(BF_NP)

        in_maps.append({
            "xpm": xpm, "wqa": wqa, "wqb": wqb, "wv": wv_pm, "wo": wo_pm,
            "bqk": bqk_np, "bv": bv_np, "cos4": cos4, "sin4": sin4,
            "maskT": maskT, "idm": idm,
        })

    trace = os.environ.get("KERNEL_TRACE", "0") == "1"
    res = run_bass_kernel_spmd(nc, in_maps, core_ids=list(range(NCORES)),
                               trace=trace)
    if res.exec_time_ns is not None:
        print(f"HW exec time: {res.exec_time_ns} ns")
        if res.instructions_and_trace is not None:
            print(f"trace: {res.instructions_and_trace[1]}")

    acc = np.zeros((B * S, D), dtype=np.float32)
    for c in range(NCORES):
        acc += res.results[c]["pout"].astype(np.float32).reshape(B * S, D)
    out = acc + b_out[None, :]
    return out.reshape(B, S, D)


# revision 43
# speedup vs baseline: 1.2093x; 1.0057x over previous
"""Bass/Trainium2 kernel for nn_AttentionLayer_68229850464552.

Full multi-head causal attention layer (QKV proj + partial RoPE + attention +
output proj), head-sharded (tensor parallel) across 8 NeuronCores. Each core
computes 2 of the 16 heads for both batch elements and the partial output
projection for its heads' feature columns; the host scales by 1 and sums the
8 partials and adds the output bias.

Matmul operands are bf16 (PE streams 2B/lane/cycle -> 1 cycle/row); fp32
accumulation in PSUM throughout.

Self-contained: hardcodes shapes from the problem spec.
"""
import os
import numpy as np
import ml_dtypes
from contextlib import ExitStack

import concourse.bass as bass
import concourse.mybir as mybir
import concourse.tile as tile
from concourse import bacc
from concourse.bass_utils import run_bass_kernel_spmd

B, S, D, H, DK = 2, 2048, 2048, 16, 128
HPC = 2                      # heads per core
NCORES = 8
DR = 32                      # rope features
SCALE = 1.0 / float(np.sqrt(DK))
CH = 512                     # x seq-chunk width for the QKV projection
NCH = S // CH                # 4
QCW = 512                    # query chunk width in attention
NQC = S // QCW               # 4
NJ = S // 128                # 16 key blocks
WV_COLS = 2 * 128            # [v_h0 | v_h1]

F32 = mybir.dt.float32
F32R = mybir.dt.float32r
BF16 = mybir.dt.bfloat16
Act = mybir.ActivationFunctionType
Alu = mybir.AluOpType
BF_NP = ml_dtypes.bfloat16

_PROG_CACHE = {}


def _build_program():
    nc = bacc.Bacc("TRN2", target_bir_lowering=False, debug=False,
                   enable_asserts=True, num_devices=NCORES)

    # all weight/const tensors are partition-major on the host so DMAs are
    # contiguous per partition (fat descriptors)
    xpm = nc.dram_tensor("xpm", [128, B, NCH, 16, CH], BF16,
                         kind="ExternalInput").ap()
    wqa = nc.dram_tensor("wqa", [128, 16, 128], BF16, kind="ExternalInput").ap()
    wqb = nc.dram_tensor("wqb", [128, 16, 384], BF16, kind="ExternalInput").ap()
    wv = nc.dram_tensor("wv", [128, 16, WV_COLS], BF16,
                        kind="ExternalInput").ap()
    wo = nc.dram_tensor("wo", [128, HPC, D], BF16, kind="ExternalInput").ap()
    bqk = nc.dram_tensor("bqk", [128, 4], F32, kind="ExternalInput").ap()
    bv = nc.dram_tensor("bv", [128, WV_COLS], F32, kind="ExternalInput").ap()
    cos4 = nc.dram_tensor("cos4", [128, S], BF16, kind="ExternalInput").ap()
    sin4 = nc.dram_tensor("sin4", [128, S], BF16, kind="ExternalInput").ap()
    maskT = nc.dram_tensor("maskT", [128, 128], BF16, kind="ExternalInput").ap()
    idm = nc.dram_tensor("idm", [128, 128], BF16, kind="ExternalInput").ap()
    pout = nc.dram_tensor("pout", [B, 16, 128, D], BF16,
                          kind="ExternalOutput").ap()

    with tile.TileContext(nc) as tc, ExitStack() as ctx:
        wpool = ctx.enter_context(tc.tile_pool(name="w", bufs=1))
        xpool = ctx.enter_context(tc.tile_pool(name="x", bufs=3))
        qkpool = ctx.enter_context(tc.tile_pool(name="qk", bufs=2))
        vpool = ctx.enter_context(tc.tile_pool(name="v", bufs=2))
        otpool = ctx.enter_context(tc.tile_pool(name="ot", bufs=2))
        ppool = ctx.enter_context(tc.tile_pool(name="p", bufs=6))
        rpool = ctx.enter_context(tc.tile_pool(name="r", bufs=2))
        opool = ctx.enter_context(tc.tile_pool(name="o", bufs=3))
        ncpool = ctx.enter_context(tc.tile_pool(name="nc", bufs=3))
        prpool = ctx.enter_context(tc.tile_pool(name="pr", bufs=2))
        scpool = ctx.enter_context(tc.tile_pool(name="sc", bufs=3, space="PSUM"))
        accpool = ctx.enter_context(tc.tile_pool(name="acc", bufs=2,
                                                 space="PSUM"))
        pjpool = ctx.enter_context(tc.tile_pool(name="pj", bufs=3,
                                                space="PSUM"))

        # ---- startup: only what chunk 0/1 need, interleaved so PE starts
        # early; everything else is deferred until after the xt1 prefetch.
        wq0_sb = wpool.tile([128, 16, 128], BF16)
        xt0 = xpool.tile([128, 16, CH], BF16, tag="xt", name="xt0")
        nc.sync.dma_start(wq0_sb[:, 0:4, :], wqa[:, 0:4, :])
        nc.sync.dma_start(xt0[:, 0:2, :], xpm[:, 0, 0, 0:2, :])
        nc.sync.dma_start(wq0_sb[:, 4:16, :], wqa[:, 4:16, :])
        nc.sync.dma_start(xt0[:, 2:4, :], xpm[:, 0, 0, 2:4, :])
        nc.sync.dma_start(xt0[:, 4:8, :], xpm[:, 0, 0, 4:8, :])
        nc.sync.dma_start(xt0[:, 8:16, :], xpm[:, 0, 0, 8:16, :])
        bqk_sb = wpool.tile([128, 4], F32)
        nc.sync.dma_start(bqk_sb[:], bqk[:])
        wqb_sb = wpool.tile([128, 16, 384], BF16)
        nc.sync.dma_start(wqb_sb[:], wqb[:])
        wv_sb = wpool.tile([128, 16, WV_COLS], BF16)
        nc.sync.dma_start(wv_sb[:], wv[:])
        ones_sb = wpool.tile([128, 128], BF16)
        nc.gpsimd.memset(ones_sb[:], 1.0)
        # ramp the PE p-state to 2.4GHz while the startup DMAs stream: the
        # HAM only reaches full clock after ~3.4us of continuous PE activity
        junk_ps = pjpool.tile([128, 512], F32, tag="pj", name="junk")
        for _ in range(36):
            nc.tensor.matmul(junk_ps[:, 0:128], ones_sb[:], ones_sb[:],
                             start=True, stop=True)

        # declared here, loaded by deferred_consts() after xt1's prefetch is
        # in the DMA queue (they are only needed tens of us into the run)
        sin4_sb = wpool.tile([128, S], BF16)
        cos4_sb = wpool.tile([128, S], BF16)
        bv_sb = wpool.tile([128, WV_COLS], F32)
        maskT_sb = wpool.tile([128, 128], BF16)
        idm_sb = wpool.tile([128, 128], BF16)
        wo_sb = wpool.tile([128, HPC, D], BF16)

        def deferred_consts():
            nc.sync.dma_start(sin4_sb[:], sin4[:])
            nc.sync.dma_start(cos4_sb[:], cos4[:])
            nc.sync.dma_start(bv_sb[:], bv[:])
            nc.sync.dma_start(maskT_sb[:], maskT[:])
            nc.sync.dma_start(idm_sb[:], idm[:])
            nc.sync.dma_start(wo_sb[:], wo[:])

        def qkv_chunk(b, c, xt, qk_sb, v_sb, split_k=False):
            cs = slice(c * CH, (c + 1) * CH)

            def qk_mms(mt, ps, kts):
                for kt in kts:
                    w = (wq0_sb[:, kt, :] if mt == 0
                         else wqb_sb[:, kt, (mt - 1) * 128:mt * 128])
                    nc.tensor.matmul(ps[:], w, xt[:, kt, :],
                                     start=(kt == 0), stop=(kt == 15))

            if split_k:
                # chunk 0 only: run every group's first kt-half before any
                # second half so the PE isn't gated on the full xt stream
                pss = []
                for mt in range(3):
                    ps = pjpool.tile([128, CH], F32, tag="pj", name="ps")
                    qk_mms(mt, ps, range(8))
                    pss.append(ps)
                qk_mms(0, pss[0], range(8, 16))
                nc.scalar.activation(qk_sb[0][:, cs], pss[0][:], Act.Identity,
                                     bias=bqk_sb[:, 0:1])
                ps3 = pjpool.tile([128, CH], F32, tag="pj", name="ps")
                qk_mms(3, ps3, range(8))
                pss.append(ps3)
                for mt in range(1, 4):
                    qk_mms(mt, pss[mt], range(8, 16))
                    nc.scalar.activation(qk_sb[mt][:, cs], pss[mt][:],
                                         Act.Identity,
                                         bias=bqk_sb[:, mt:mt + 1])
            else:
                for mt in range(4):
                    ps = pjpool.tile([128, CH], F32, tag="pj", name="ps")
                    qk_mms(mt, ps, range(16))
                    nc.scalar.activation(qk_sb[mt][:, cs], ps[:], Act.Identity,
                                         bias=bqk_sb[:, mt:mt + 1])

            # RoPE, packed: all 4 tensors' rope rows on 128 partitions.
            # out = q·cos + shuf(q)·sin  (sin sign-folded on host)
            shuf = rpool.tile([128, CH], BF16, tag="shuf", name="shuf")
            ra = rpool.tile([128, CH], BF16, tag="ra", name="ra")
            for t in range(4):
                nc.sync.dma_start(shuf[32 * t:32 * t + 16, :],
                                  qk_sb[t][16:32, cs])
                nc.sync.dma_start(shuf[32 * t + 16:32 * t + 32, :],
                                  qk_sb[t][0:16, cs])
                nc.sync.dma_start(ra[32 * t:32 * t + 32, :],
                                  qk_sb[t][0:32, cs])
            tmp = rpool.tile([128, CH], F32, tag="rtmp", name="tmp")
            nc.vector.tensor_tensor(tmp[:], shuf[:], sin4_sb[:, cs], Alu.mult)
            ro = rpool.tile([128, CH], BF16, tag="ro", name="ro")
            nc.vector.tensor_tensor(ro[:], ra[:], cos4_sb[:, cs], Alu.mult)
            nc.vector.tensor_tensor(ro[:], ro[:], tmp[:], Alu.add)
            for t in range(4):
                nc.sync.dma_start(qk_sb[t][0:32, cs], ro[32 * t:32 * t + 32, :])

            # V projection for this chunk ([seq, feat] layout)
            for s2 in range(4):
                psv = pjpool.tile([128, WV_COLS], F32, tag="pj", name="psv")
                for kt in range(16):
                    nc.tensor.matmul(
                        psv[:], xt[:, kt, s2 * 128:(s2 + 1) * 128],
                        wv_sb[:, kt, :], start=(kt == 0), stop=(kt == 15))
                nc.vector.tensor_tensor(v_sb[:, c * 4 + s2, :], psv[:],
                                        bv_sb[:], Alu.add)

        pending_fin = [None]

        def attn_unit(b, qc, h, qk_sb, v_sb, ot_sb, pe_sums=False, last=False):
            """Returns deferred norm closure.

            pe_sums: softmax denominators via per-j ones-matmuls on the PE
            (used where DVE/GpSimd partial accumulation doesn't work: qc==0
            has diag blocks as first touches, and the final unit can't absorb
            the partial-chain latency in its tail).  Otherwise exp tiles are
            accumulated elementwise on DVE/GpSimd (split by j parity) and
            reduced with two fp32r ones-matmuls at the end.
            """
            jmax = 4 * qc + 3
            otps = accpool.tile([128, QCW], F32, tag="acc", name="otps")
            sums = accpool.tile([128, QCW], F32, tag="acc", name="sums")
            if not pe_sums:
                part = [prpool.tile([128, QCW], BF16, tag=f"part{e}",
                                    name=f"part{e}") for e in range(2)]

            def emit_score(j):
                c0 = (j - 4 * qc) * 128 if j >= 4 * qc else 0
                diag = j >= 4 * qc
                sps = scpool.tile([128, QCW], F32, tag="sc", name="sps")
                nc.tensor.matmul(
                    sps[:, c0:QCW], qk_sb[2 + h][:, j * 128:(j + 1) * 128],
                    qk_sb[h][:, qc * QCW + c0:(qc + 1) * QCW],
                    start=True, stop=not diag)
                if diag:
                    # add -1e4 above the diagonal of the diag subblock
                    nc.tensor.matmul(
                        sps[:, c0:c0 + 128], maskT_sb[:], idm_sb[:],
                        start=False, stop=True)
                return sps

            def emit_consume(j, sps):
                c0 = (j - 4 * qc) * 128 if j >= 4 * qc else 0
                pt = ppool.tile([128, QCW], BF16, tag="pt", name="pt")
                nc.scalar.activation(pt[:, c0:QCW], sps[:, c0:QCW],
                                     Act.Exp, scale=SCALE)
                nc.tensor.matmul(
                    otps[:, c0:QCW],
                    v_sb[:, j, 128 * h:128 * (h + 1)],
                    pt[:, c0:QCW], start=(j == 0), stop=(j == jmax))
                if pe_sums:
                    nc.tensor.matmul(
                        sums[:, c0:QCW], ones_sb[:],
                        pt[:, c0:QCW], start=(j == 0), stop=(j == jmax))
                else:
                    eng = nc.vector if j % 2 == 0 else nc.gpsimd
                    tgt = part[j % 2]
                    if j < 2:
                        # j<2 are full blocks whenever qc>=1
                        nc.vector.tensor_copy(tgt[:], pt[:])
                    else:
                        with nc.allow_low_precision(
                                "bf16 softmax-denominator partials"):
                            eng.tensor_tensor(tgt[:, c0:QCW], tgt[:, c0:QCW],
                                              pt[:, c0:QCW], Alu.add)

            # 2-deep score lookahead: exp(j) gets ~2 blocks of PE work to
            # hide behind before pv(j) needs it (scpool holds 3 banks)
            sq = [emit_score(0), emit_score(1)]
            # the previous unit's finishing sequence (reduce MMs waiting on
            # the slow partial-add chains, copy, recip, norm) is emitted
            # HERE so the PE reaches this unit's scores before stalling on it
            if pending_fin[0] is not None:
                pending_fin[0]()
                pending_fin[0] = None
            for j in range(2, jmax + 1):
                sq.append(emit_score(j))
                emit_consume(j - 2, sq.pop(0))
            emit_consume(jmax - 1, sq.pop(0))
            emit_consume(jmax, sq.pop(0))

            otr = ncpool.tile([128, QCW], F32, tag="otr", name="otr")
            rc = ncpool.tile([128, QCW], F32, tag="rc", name="rc")

            def fin():
                if not pe_sums:
                    nc.tensor.matmul(sums[:], ones_sb[:], part[0][:],
                                     start=True, stop=False)
                    nc.tensor.matmul(sums[:], ones_sb[:], part[1][:],
                                     start=False, stop=True)
                if last:
                    # slice the norm so the final outproj starts per-sblk
                    for s in range(4):
                        sl = slice(s * 128, (s + 1) * 128)
                        nc.scalar.activation(otr[:, sl], otps[:, sl], Act.Copy)
                        nc.vector.reciprocal_approx_fast(rc[:, sl],
                                                         sums[:, sl])
                        nc.gpsimd.tensor_tensor(
                            ot_sb[:, h, qc * QCW + s * 128:
                                  qc * QCW + (s + 1) * 128],
                            otr[:, sl], rc[:, sl], Alu.mult)
                    return
                nc.vector.tensor_copy(otr[:], otps[:])
                nc.vector.reciprocal_approx_fast(rc[:], sums[:])
                nc.gpsimd.tensor_tensor(ot_sb[:, h, qc * QCW:(qc + 1) * QCW],
                                        otr[:], rc[:], Alu.mult)

            if last:
                fin()
            else:
                pending_fin[0] = fin

        def outproj(b, qc, ot_sb):
            for sblk in range(4 * qc, 4 * qc + 4):
                po = opool.tile([128, D], BF16, tag="po", name="po")
                for n in range(4):
                    psc = pjpool.tile([128, 512], F32, tag="pj", name="psc")
                    for kt in range(HPC):
                        nc.tensor.matmul(
                            psc[:], ot_sb[:, kt, sblk * 128:(sblk + 1) * 128],
                            wo_sb[:, kt, n * 512:(n + 1) * 512],
                            start=(kt == 0), stop=(kt == 1))
                    if n % 2 == 0:
                        nc.vector.tensor_copy(po[:, n * 512:(n + 1) * 512],
                                              psc[:])
                    else:
                        nc.scalar.activation(po[:, n * 512:(n + 1) * 512],
                                             psc[:], Act.Copy)
                    if n % 2 == 1:
                        nc.sync.dma_start(
                            pout[b, sblk, :, (n - 1) * 512:(n + 1) * 512],
                            po[:, (n - 1) * 512:(n + 1) * 512])

        # ---------------- batch 0 QKV ----------------
        qk0 = [qkpool.tile([128, S], BF16, tag=f"qk{t}", name=f"qk{t}_b0")
               for t in range(4)]
        v0 = vpool.tile([128, NJ, WV_COLS], BF16, tag="v", name="v_b0")
        xt_cur = xt0
        for c in range(NCH):
            if c + 1 < NCH:
                xt_next = xpool.tile([128, 16, CH], BF16, tag="xt", name="xtn")
                nc.sync.dma_start(xt_next[:], xpm[:, 0, c + 1, :, :])
            else:
                xt_next = xpool.tile([128, 16, CH], BF16, tag="xt", name="xtn")
                nc.sync.dma_start(xt_next[:], xpm[:, 1, 0, :, :])
            if c == 0:
                deferred_consts()
            qkv_chunk(0, c, xt_cur, qk0, v0)
            xt_cur = xt_next

        # ---- attention: b0 units, b1 QKV, and b1 units interleaved ----
        qk1 = [qkpool.tile([128, S], BF16, tag=f"qk{t}", name=f"qk{t}_b1")
               for t in range(4)]
        v1 = vpool.tile([128, NJ, WV_COLS], BF16, tag="v", name="v_b1")
        ot0 = otpool.tile([128, HPC, S], BF16, tag="ot", name="ot_b0")
        ot1 = otpool.tile([128, HPC, S], BF16, tag="ot", name="ot_b1")
        def unit(b, qc, h, pe_sums=False, last=False):
            attn_unit(b, qc, h, qk1 if b else qk0, v1 if b else v0,
                      ot1 if b else ot0, pe_sums=pe_sums, last=last)

        for qc in range(NQC):
            for h in range(HPC):
                unit(0, qc, h, pe_sums=(qc == 0))
            # batch 1 chunk qc QKV goes here to fill PE bubbles
            if qc + 1 < NCH:
                xt_next = xpool.tile([128, 16, CH], BF16, tag="xt", name="xtn")
                nc.sync.dma_start(xt_next[:], xpm[:, 1, qc + 1, :, :])
            qkv_chunk(1, qc, xt_cur, qk1, v1)
            xt_cur = xt_next if qc + 1 < NCH else None
            if qc >= 1:
                for h in range(HPC):
                    unit(1, qc - 1, h, pe_sums=(qc - 1 == 0))
            if qc == 1:
                outproj(0, 0, ot0)
            if qc >= 2:
                outproj(0, qc - 1, ot0)
                outproj(1, qc - 2, ot1)
        outproj(0, 3, ot0)
        unit(1, 3, 0)
        outproj(1, 2, ot1)
        unit(1, 3, 1, pe_sums=True, last=True)
        outproj(1, 3, ot1)

    nc.compile()
    return nc


def kernel(x, W_qkv, b_qkv, W_out, b_out):
    x = np.asarray(x, dtype=np.float32)
    W_qkv = np.asarray(W_qkv, dtype=np.float32)
    b_qkv = np.asarray(b_qkv, dtype=np.float32)
    W_out = np.asarray(W_out, dtype=np.float32)
    b_out = np.asarray(b_out, dtype=np.float32)

    if "prog" not in _PROG_CACHE:
        _PROG_CACHE["prog"] = _build_program()
    nc = _PROG_CACHE["prog"]

    # x -> [p, b, c, kt, s'] partition-major layout
    xpm = np.ascontiguousarray(
        x.reshape(B, NCH, CH, 16, 128).transpose(4, 0, 1, 3, 2)
    ).astype(BF_NP)

    i = np.arange(16, dtype=np.float64)
    theta = 1.0 / (10000.0 ** ((2.0 * i) / DR))
    s_idx = np.arange(S, dtype=np.float64)
    idx = s_idx[:, None] * theta[None, :]          # [S, 16]
    idx2 = np.concatenate([idx, idx], axis=1)      # [S, 32]
    cosT = np.cos(idx2).T.astype(np.float32)       # [32, S]
    sinT = np.sin(idx2).T.astype(np.float32)
    sinT[0:16, :] *= -1.0      # sign of rot = [-q[16:32], q[0:16]] folded in
    cos4 = np.ascontiguousarray(np.tile(cosT, (4, 1)).astype(BF_NP))  # [128,S]
    sin4 = np.ascontiguousarray(np.tile(sinT, (4, 1)).astype(BF_NP))

    maskT = np.triu(np.full((128, 128), -10000.0, dtype=np.float32),
                    1).astype(BF_NP)
    idm = np.eye(128, dtype=np.float32).astype(BF_NP)

    def part_major(w_cols):
        # w_cols: [D, M] -> [128, D//128, M] partition-major
        M = w_cols.shape[1]
        return np.ascontiguousarray(
            w_cols.reshape(16, 128, M).transpose(1, 0, 2))

    in_maps = []
    for c in range(NCORES):
        heads = [HPC * c, HPC * c + 1]
        qw, kw, vw, qb, kb, vb = [], [], [], [], [], []
        for hh in heads:
            base = 3 * DK * hh
            qw.append(W_qkv[base:base + 128])
            kw.append(W_qkv[base + 128:base + 256])
            vw.append(W_qkv[base + 256:base + 384])
            qb.append(b_qkv[base:base + 128])
            kb.append(b_qkv[base + 128:base + 256])
            vb.append(b_qkv[base + 256:base + 384])

        wq_full = np.concatenate([qw[0], qw[1], kw[0], kw[1]], axis=0).T
        wq_pm = part_major(wq_full).astype(BF_NP)        # [128, 16, 512]
        wqa = np.ascontiguousarray(wq_pm[:, :, 0:128])
        wqb = np.ascontiguousarray(wq_pm[:, :, 128:512])

        wv_full = np.concatenate([vw[0], vw[1]], axis=0).T   # [D, 256]
        wv_pm = np.ascontiguousarray(part_major(wv_full).astype(BF_NP))

        bv_np = np.concatenate([vb[0], vb[1]])[None, :]
        bv_np = np.ascontiguousarray(np.repeat(bv_np, 128, axis=0))

        bqk_np = np.zeros((128, 4), dtype=np.float32)
        bqk_np[:, 0] = qb[0]
        bqk_np[:, 1] = qb[1]
        bqk_np[:, 2] = kb[0]
        bqk_np[:, 3] = kb[1]

        wo_full = W_out[:, HPC * DK * c: HPC * DK * (c + 1)].T   # [256, D]
        wo_pm = np.ascontiguousarray(
            wo_full.reshape(2, 128, D).transpose(1, 0, 2)).astype(BF_NP)

        in_maps.append({
            "xpm": xpm, "wqa": wqa, "wqb": wqb, "wv": wv_pm, "wo": wo_pm,
            "bqk": bqk_np, "bv": bv_np, "cos4": cos4, "sin4": sin4,
            "maskT": maskT, "idm": idm,
        })

    trace = os.environ.get("KERNEL_TRACE", "0") == "1"
    res = run_bass_kernel_spmd(nc, in_maps, core_ids=list(range(NCORES)),
                               trace=trace)
    if res.exec_time_ns is not None:
        print(f"HW exec time: {res.exec_time_ns} ns")
        if res.instructions_and_trace is not None:
            print(f"trace: {res.instructions_and_trace[1]}")

    acc = np.zeros((B * S, D), dtype=np.float32)
    for c in range(NCORES):
        acc += res.results[c]["pout"].astype(np.float32).reshape(B * S, D)
    out = acc + b_out[None, :]
    return out.reshape(B, S, D)


# revision 44
# speedup vs baseline: 1.2112x; 1.0016x over previous
"""Bass/Trainium2 kernel for nn_AttentionLayer_68229850464552.

Full multi-head causal attention layer (QKV proj + partial RoPE + attention +
output proj), head-sharded (tensor parallel) across 8 NeuronCores. Each core
computes 2 of the 16 heads for both batch elements and the partial output
projection for its heads' feature columns; the host scales by 1 and sums the
8 partials and adds the output bias.

Matmul operands are bf16 (PE streams 2B/lane/cycle -> 1 cycle/row); fp32
accumulation in PSUM throughout.

Self-contained: hardcodes shapes from the problem spec.
"""
import os
import numpy as np
import ml_dtypes
from contextlib import ExitStack

import concourse.bass as bass
import concourse.mybir as mybir
import concourse.tile as tile
from concourse import bacc
from concourse.bass_utils import run_bass_kernel_spmd

B, S, D, H, DK = 2, 2048, 2048, 16, 128
HPC = 2                      # heads per core
NCORES = 8
DR = 32                      # rope features
SCALE = 1.0 / float(np.sqrt(DK))
CH = 512                     # x seq-chunk width for the QKV projection
NCH = S // CH                # 4
QCW = 512                    # query chunk width in attention
NQC = S // QCW               # 4
NJ = S // 128                # 16 key blocks
WV_COLS = 2 * 128            # [v_h0 | v_h1]

F32 = mybir.dt.float32
F32R = mybir.dt.float32r
BF16 = mybir.dt.bfloat16
Act = mybir.ActivationFunctionType
Alu = mybir.AluOpType
BF_NP = ml_dtypes.bfloat16

_PROG_CACHE = {}


def _build_program():
    nc = bacc.Bacc("TRN2", target_bir_lowering=False, debug=False,
                   enable_asserts=True, num_devices=NCORES)

    # all weight/const tensors are partition-major on the host so DMAs are
    # contiguous per partition (fat descriptors)
    xpm = nc.dram_tensor("xpm", [128, B, NCH, 16, CH], BF16,
                         kind="ExternalInput").ap()
    wqa = nc.dram_tensor("wqa", [128, 16, 128], BF16, kind="ExternalInput").ap()
    wqb = nc.dram_tensor("wqb", [128, 16, 384], BF16, kind="ExternalInput").ap()
    wv = nc.dram_tensor("wv", [128, 16, WV_COLS], BF16,
                        kind="ExternalInput").ap()
    wo = nc.dram_tensor("wo", [128, HPC, D], BF16, kind="ExternalInput").ap()
    bqk = nc.dram_tensor("bqk", [128, 4], F32, kind="ExternalInput").ap()
    bv = nc.dram_tensor("bv", [128, WV_COLS], F32, kind="ExternalInput").ap()
    cos4 = nc.dram_tensor("cos4", [128, S], BF16, kind="ExternalInput").ap()
    sin4 = nc.dram_tensor("sin4", [128, S], BF16, kind="ExternalInput").ap()
    maskT = nc.dram_tensor("maskT", [128, 128], BF16, kind="ExternalInput").ap()
    idm = nc.dram_tensor("idm", [128, 128], BF16, kind="ExternalInput").ap()
    pout = nc.dram_tensor("pout", [B, 16, 128, D], BF16,
                          kind="ExternalOutput").ap()

    with tile.TileContext(nc) as tc, ExitStack() as ctx:
        wpool = ctx.enter_context(tc.tile_pool(name="w", bufs=1))
        xpool = ctx.enter_context(tc.tile_pool(name="x", bufs=3))
        qkpool = ctx.enter_context(tc.tile_pool(name="qk", bufs=2))
        vpool = ctx.enter_context(tc.tile_pool(name="v", bufs=2))
        otpool = ctx.enter_context(tc.tile_pool(name="ot", bufs=2))
        ppool = ctx.enter_context(tc.tile_pool(name="p", bufs=6))
        rpool = ctx.enter_context(tc.tile_pool(name="r", bufs=2))
        opool = ctx.enter_context(tc.tile_pool(name="o", bufs=3))
        ncpool = ctx.enter_context(tc.tile_pool(name="nc", bufs=3))
        prpool = ctx.enter_context(tc.tile_pool(name="pr", bufs=2))
        scpool = ctx.enter_context(tc.tile_pool(name="sc", bufs=3, space="PSUM"))
        accpool = ctx.enter_context(tc.tile_pool(name="acc", bufs=2,
                                                 space="PSUM"))
        pjpool = ctx.enter_context(tc.tile_pool(name="pj", bufs=3,
                                                space="PSUM"))

        # ---- startup: only what chunk 0/1 need, interleaved so PE starts
        # early; everything else is deferred until after the xt1 prefetch.
        wq0_sb = wpool.tile([128, 16, 128], BF16)
        xt0 = xpool.tile([128, 16, CH], BF16, tag="xt", name="xt0")
        nc.sync.dma_start(wq0_sb[:, 0:4, :], wqa[:, 0:4, :])
        nc.sync.dma_start(xt0[:, 0:2, :], xpm[:, 0, 0, 0:2, :])
        nc.sync.dma_start(wq0_sb[:, 4:16, :], wqa[:, 4:16, :])
        nc.sync.dma_start(xt0[:, 2:4, :], xpm[:, 0, 0, 2:4, :])
        nc.sync.dma_start(xt0[:, 4:8, :], xpm[:, 0, 0, 4:8, :])
        nc.sync.dma_start(xt0[:, 8:16, :], xpm[:, 0, 0, 8:16, :])
        bqk_sb = wpool.tile([128, 4], F32)
        nc.sync.dma_start(bqk_sb[:], bqk[:])
        wqb_sb = wpool.tile([128, 16, 384], BF16)
        nc.sync.dma_start(wqb_sb[:, :, 0:128], wqb[:, :, 0:128])
        nc.sync.dma_start(wqb_sb[:, :, 128:384], wqb[:, :, 128:384])
        wv_sb = wpool.tile([128, 16, WV_COLS], BF16)
        nc.sync.dma_start(wv_sb[:], wv[:])
        ones_sb = wpool.tile([128, 128], BF16)
        nc.gpsimd.memset(ones_sb[:], 1.0)
        # ramp the PE p-state to 2.4GHz while the startup DMAs stream: the
        # HAM only reaches full clock after ~3.4us of continuous PE activity
        junk_ps = pjpool.tile([128, 512], F32, tag="pj", name="junk")
        for _ in range(36):
            nc.tensor.matmul(junk_ps[:, 0:128], ones_sb[:], ones_sb[:],
                             start=True, stop=True)

        # declared here, loaded by deferred_consts() after xt1's prefetch is
        # in the DMA queue (they are only needed tens of us into the run)
        sin4_sb = wpool.tile([128, S], BF16)
        cos4_sb = wpool.tile([128, S], BF16)
        bv_sb = wpool.tile([128, WV_COLS], F32)
        maskT_sb = wpool.tile([128, 128], BF16)
        idm_sb = wpool.tile([128, 128], BF16)
        wo_sb = wpool.tile([128, HPC, D], BF16)

        def deferred_consts():
            nc.sync.dma_start(sin4_sb[:], sin4[:])
            nc.sync.dma_start(cos4_sb[:], cos4[:])
            nc.sync.dma_start(bv_sb[:], bv[:])
            nc.sync.dma_start(maskT_sb[:], maskT[:])
            nc.sync.dma_start(idm_sb[:], idm[:])
            nc.sync.dma_start(wo_sb[:], wo[:])

        def qkv_chunk(b, c, xt, qk_sb, v_sb, split_k=False):
            cs = slice(c * CH, (c + 1) * CH)

            def qk_mms(mt, ps, kts):
                for kt in kts:
                    w = (wq0_sb[:, kt, :] if mt == 0
                         else wqb_sb[:, kt, (mt - 1) * 128:mt * 128])
                    nc.tensor.matmul(ps[:], w, xt[:, kt, :],
                                     start=(kt == 0), stop=(kt == 15))

            if split_k:
                # chunk 0 only: run every group's first kt-half before any
                # second half so the PE isn't gated on the full xt stream
                pss = []
                for mt in range(3):
                    ps = pjpool.tile([128, CH], F32, tag="pj", name="ps")
                    qk_mms(mt, ps, range(8))
                    pss.append(ps)
                qk_mms(0, pss[0], range(8, 16))
                nc.scalar.activation(qk_sb[0][:, cs], pss[0][:], Act.Identity,
                                     bias=bqk_sb[:, 0:1])
                ps3 = pjpool.tile([128, CH], F32, tag="pj", name="ps")
                qk_mms(3, ps3, range(8))
                pss.append(ps3)
                for mt in range(1, 4):
                    qk_mms(mt, pss[mt], range(8, 16))
                    nc.scalar.activation(qk_sb[mt][:, cs], pss[mt][:],
                                         Act.Identity,
                                         bias=bqk_sb[:, mt:mt + 1])
            else:
                for mt in range(4):
                    ps = pjpool.tile([128, CH], F32, tag="pj", name="ps")
                    qk_mms(mt, ps, range(16))
                    nc.scalar.activation(qk_sb[mt][:, cs], ps[:], Act.Identity,
                                         bias=bqk_sb[:, mt:mt + 1])

            # RoPE, packed: all 4 tensors' rope rows on 128 partitions.
            # out = q·cos + shuf(q)·sin  (sin sign-folded on host)
            shuf = rpool.tile([128, CH], BF16, tag="shuf", name="shuf")
            ra = rpool.tile([128, CH], BF16, tag="ra", name="ra")
            for t in range(4):
                nc.sync.dma_start(shuf[32 * t:32 * t + 16, :],
                                  qk_sb[t][16:32, cs])
                nc.sync.dma_start(shuf[32 * t + 16:32 * t + 32, :],
                                  qk_sb[t][0:16, cs])
                nc.sync.dma_start(ra[32 * t:32 * t + 32, :],
                                  qk_sb[t][0:32, cs])
            tmp = rpool.tile([128, CH], F32, tag="rtmp", name="tmp")
            nc.vector.tensor_tensor(tmp[:], shuf[:], sin4_sb[:, cs], Alu.mult)
            ro = rpool.tile([128, CH], BF16, tag="ro", name="ro")
            nc.vector.tensor_tensor(ro[:], ra[:], cos4_sb[:, cs], Alu.mult)
            nc.vector.tensor_tensor(ro[:], ro[:], tmp[:], Alu.add)
            for t in range(4):
                nc.sync.dma_start(qk_sb[t][0:32, cs], ro[32 * t:32 * t + 32, :])

            # V projection for this chunk ([seq, feat] layout)
            for s2 in range(4):
                psv = pjpool.tile([128, WV_COLS], F32, tag="pj", name="psv")
                for kt in range(16):
                    nc.tensor.matmul(
                        psv[:], xt[:, kt, s2 * 128:(s2 + 1) * 128],
                        wv_sb[:, kt, :], start=(kt == 0), stop=(kt == 15))
                nc.vector.tensor_tensor(v_sb[:, c * 4 + s2, :], psv[:],
                                        bv_sb[:], Alu.add)

        pending_fin = [None]

        def attn_unit(b, qc, h, qk_sb, v_sb, ot_sb, pe_sums=False, last=False):
            """Returns deferred norm closure.

            pe_sums: softmax denominators via per-j ones-matmuls on the PE
            (used where DVE/GpSimd partial accumulation doesn't work: qc==0
            has diag blocks as first touches, and the final unit can't absorb
            the partial-chain latency in its tail).  Otherwise exp tiles are
            accumulated elementwise on DVE/GpSimd (split by j parity) and
            reduced with two fp32r ones-matmuls at the end.
            """
            jmax = 4 * qc + 3
            otps = accpool.tile([128, QCW], F32, tag="acc", name="otps")
            sums = accpool.tile([128, QCW], F32, tag="acc", name="sums")
            if not pe_sums:
                part = [prpool.tile([128, QCW], BF16, tag=f"part{e}",
                                    name=f"part{e}") for e in range(2)]

            def emit_score(j):
                c0 = (j - 4 * qc) * 128 if j >= 4 * qc else 0
                diag = j >= 4 * qc
                sps = scpool.tile([128, QCW], F32, tag="sc", name="sps")
                nc.tensor.matmul(
                    sps[:, c0:QCW], qk_sb[2 + h][:, j * 128:(j + 1) * 128],
                    qk_sb[h][:, qc * QCW + c0:(qc + 1) * QCW],
                    start=True, stop=not diag)
                if diag:
                    # add -1e4 above the diagonal of the diag subblock
                    nc.tensor.matmul(
                        sps[:, c0:c0 + 128], maskT_sb[:], idm_sb[:],
                        start=False, stop=True)
                return sps

            def emit_consume(j, sps):
                c0 = (j - 4 * qc) * 128 if j >= 4 * qc else 0
                pt = ppool.tile([128, QCW], BF16, tag="pt", name="pt")
                nc.scalar.activation(pt[:, c0:QCW], sps[:, c0:QCW],
                                     Act.Exp, scale=SCALE)
                nc.tensor.matmul(
                    otps[:, c0:QCW],
                    v_sb[:, j, 128 * h:128 * (h + 1)],
                    pt[:, c0:QCW], start=(j == 0), stop=(j == jmax))
                if pe_sums:
                    nc.tensor.matmul(
                        sums[:, c0:QCW], ones_sb[:],
                        pt[:, c0:QCW], start=(j == 0), stop=(j == jmax))
                else:
                    eng = nc.vector if j % 2 == 0 else nc.gpsimd
                    tgt = part[j % 2]
                    if j < 2:
                        # j<2 are full blocks whenever qc>=1
                        nc.vector.tensor_copy(tgt[:], pt[:])
                    else:
                        with nc.allow_low_precision(
                                "bf16 softmax-denominator partials"):
                            eng.tensor_tensor(tgt[:, c0:QCW], tgt[:, c0:QCW],
                                              pt[:, c0:QCW], Alu.add)

            # 2-deep score lookahead: exp(j) gets ~2 blocks of PE work to
            # hide behind before pv(j) needs it (scpool holds 3 banks)
            sq = [emit_score(0), emit_score(1)]
            # the previous unit's finishing sequence (reduce MMs waiting on
            # the slow partial-add chains, copy, recip, norm) is emitted
            # HERE so the PE reaches this unit's scores before stalling on it
            if pending_fin[0] is not None:
                pending_fin[0]()
                pending_fin[0] = None
            for j in range(2, jmax + 1):
                sq.append(emit_score(j))
                emit_consume(j - 2, sq.pop(0))
            emit_consume(jmax - 1, sq.pop(0))
            emit_consume(jmax, sq.pop(0))

            otr = ncpool.tile([128, QCW], F32, tag="otr", name="otr")
            rc = ncpool.tile([128, QCW], F32, tag="rc", name="rc")

            def fin():
                if not pe_sums:
                    nc.tensor.matmul(sums[:], ones_sb[:], part[0][:],
                                     start=True, stop=False)
                    nc.tensor.matmul(sums[:], ones_sb[:], part[1][:],
                                     start=False, stop=True)
                if last:
                    # slice the norm so the final outproj starts per-sblk
                    for s in range(4):
                        sl = slice(s * 128, (s + 1) * 128)
                        nc.scalar.activation(otr[:, sl], otps[:, sl], Act.Copy)
                        nc.vector.reciprocal_approx_fast(rc[:, sl],
                                                         sums[:, sl])
                        nc.gpsimd.tensor_tensor(
                            ot_sb[:, h, qc * QCW + s * 128:
                                  qc * QCW + (s + 1) * 128],
                            otr[:, sl], rc[:, sl], Alu.mult)
                    return
                nc.vector.tensor_copy(otr[:], otps[:])
                nc.vector.reciprocal_approx_fast(rc[:], sums[:])
                nc.gpsimd.tensor_tensor(ot_sb[:, h, qc * QCW:(qc + 1) * QCW],
                                        otr[:], rc[:], Alu.mult)

            if last:
                fin()
            else:
                pending_fin[0] = fin

        def outproj(b, qc, ot_sb):
            for sblk in range(4 * qc, 4 * qc + 4):
                po = opool.tile([128, D], BF16, tag="po", name="po")
                for n in range(4):
                    psc = pjpool.tile([128, 512], F32, tag="pj", name="psc")
                    for kt in range(HPC):
                        nc.tensor.matmul(
                            psc[:], ot_sb[:, kt, sblk * 128:(sblk + 1) * 128],
                            wo_sb[:, kt, n * 512:(n + 1) * 512],
                            start=(kt == 0), stop=(kt == 1))
                    if n % 2 == 0:
                        nc.vector.tensor_copy(po[:, n * 512:(n + 1) * 512],
                                              psc[:])
                    else:
                        nc.scalar.activation(po[:, n * 512:(n + 1) * 512],
                                             psc[:], Act.Copy)
                    if n % 2 == 1:
                        nc.sync.dma_start(
                            pout[b, sblk, :, (n - 1) * 512:(n + 1) * 512],
                            po[:, (n - 1) * 512:(n + 1) * 512])

        # ---------------- batch 0 QKV ----------------
        qk0 = [qkpool.tile([128, S], BF16, tag=f"qk{t}", name=f"qk{t}_b0")
               for t in range(4)]
        v0 = vpool.tile([128, NJ, WV_COLS], BF16, tag="v", name="v_b0")
        xt_cur = xt0
        for c in range(NCH):
            if c + 1 < NCH:
                xt_next = xpool.tile([128, 16, CH], BF16, tag="xt", name="xtn")
                nc.sync.dma_start(xt_next[:], xpm[:, 0, c + 1, :, :])
            else:
                xt_next = xpool.tile([128, 16, CH], BF16, tag="xt", name="xtn")
                nc.sync.dma_start(xt_next[:], xpm[:, 1, 0, :, :])
            if c == 0:
                deferred_consts()
            qkv_chunk(0, c, xt_cur, qk0, v0)
            xt_cur = xt_next

        # ---- attention: b0 units, b1 QKV, and b1 units interleaved ----
        qk1 = [qkpool.tile([128, S], BF16, tag=f"qk{t}", name=f"qk{t}_b1")
               for t in range(4)]
        v1 = vpool.tile([128, NJ, WV_COLS], BF16, tag="v", name="v_b1")
        ot0 = otpool.tile([128, HPC, S], BF16, tag="ot", name="ot_b0")
        ot1 = otpool.tile([128, HPC, S], BF16, tag="ot", name="ot_b1")
        def unit(b, qc, h, pe_sums=False, last=False):
            attn_unit(b, qc, h, qk1 if b else qk0, v1 if b else v0,
                      ot1 if b else ot0, pe_sums=pe_sums, last=last)

        for qc in range(NQC):
            for h in range(HPC):
                unit(0, qc, h, pe_sums=(qc == 0))
            # batch 1 chunk qc QKV goes here to fill PE bubbles
            if qc + 1 < NCH:
                xt_next = xpool.tile([128, 16, CH], BF16, tag="xt", name="xtn")
                nc.sync.dma_start(xt_next[:], xpm[:, 1, qc + 1, :, :])
            qkv_chunk(1, qc, xt_cur, qk1, v1)
            xt_cur = xt_next if qc + 1 < NCH else None
            if qc >= 1:
                for h in range(HPC):
                    unit(1, qc - 1, h, pe_sums=(qc - 1 == 0))
            if qc == 1:
                outproj(0, 0, ot0)
            if qc >= 2:
                outproj(0, qc - 1, ot0)
                outproj(1, qc - 2, ot1)
        outproj(0, 3, ot0)
        unit(1, 3, 0)
        outproj(1, 2, ot1)
        unit(1, 3, 1, pe_sums=True, last=True)
        outproj(1, 3, ot1)

    nc.compile()
    return nc


def kernel(x, W_qkv, b_qkv, W_out, b_out):
    x = np.asarray(x, dtype=np.float32)
    W_qkv = np.asarray(W_qkv, dtype=np.float32)
    b_qkv = np.asarray(b_qkv, dtype=np.float32)
    W_out = np.asarray(W_out, dtype=np.float32)
    b_out = np.asarray(b_out, dtype=np.float32)

    if "prog" not in _PROG_CACHE:
        _PROG_CACHE["prog"] = _build_program()
    nc = _PROG_CACHE["prog"]

    # x -> [p, b, c, kt, s'] partition-major layout
    xpm = np.ascontiguousarray(
        x.reshape(B, NCH, CH, 16, 128).transpose(4, 0, 1, 3, 2)
    ).astype(BF_NP)

    i = np.arange(16, dtype=np.float64)
    theta = 1.0 / (10000.0 ** ((2.0 * i) / DR))
    s_idx = np.arange(S, dtype=np.float64)
    idx = s_idx[:, None] * theta[None, :]          # [S, 16]
    idx2 = np.concatenate([idx, idx], axis=1)      # [S, 32]
    cosT = np.cos(idx2).T.astype(np.float32)       # [32, S]
    sinT = np.sin(idx2).T.astype(np.float32)
    sinT[0:16, :] *= -1.0      # sign of rot = [-q[16:32], q[0:16]] folded in
    cos4 = np.ascontiguousarray(np.tile(cosT, (4, 1)).astype(BF_NP))  # [128,S]
    sin4 = np.ascontiguousarray(np.tile(sinT, (4, 1)).astype(BF_NP))

    maskT = np.triu(np.full((128, 128), -10000.0, dtype=np.float32),
                    1).astype(BF_NP)
    idm = np.eye(128, dtype=np.float32).astype(BF_NP)

    def part_major(w_cols):
        # w_cols: [D, M] -> [128, D//128, M] partition-major
        M = w_cols.shape[1]
        return np.ascontiguousarray(
            w_cols.reshape(16, 128, M).transpose(1, 0, 2))

    in_maps = []
    for c in range(NCORES):
        heads = [HPC * c, HPC * c + 1]
        qw, kw, vw, qb, kb, vb = [], [], [], [], [], []
        for hh in heads:
            base = 3 * DK * hh
            qw.append(W_qkv[base:base + 128])
            kw.append(W_qkv[base + 128:base + 256])
            vw.append(W_qkv[base + 256:base + 384])
            qb.append(b_qkv[base:base + 128])
            kb.append(b_qkv[base + 128:base + 256])
            vb.append(b_qkv[base + 256:base + 384])

        wq_full = np.concatenate([qw[0], qw[1], kw[0], kw[1]], axis=0).T
        wq_pm = part_major(wq_full).astype(BF_NP)        # [128, 16, 512]
        wqa = np.ascontiguousarray(wq_pm[:, :, 0:128])
        wqb = np.ascontiguousarray(wq_pm[:, :, 128:512])

        wv_full = np.concatenate([vw[0], vw[1]], axis=0).T   # [D, 256]
        wv_pm = np.ascontiguousarray(part_major(wv_full).astype(BF_NP))

        bv_np = np.concatenate([vb[0], vb[1]])[None, :]
        bv_np = np.ascontiguousarray(np.repeat(bv_np, 128, axis=0))

        bqk_np = np.zeros((128, 4), dtype=np.float32)
        bqk_np[:, 0] = qb[0]
        bqk_np[:, 1] = qb[1]
        bqk_np[:, 2] = kb[0]
        bqk_np[:, 3] = kb[1]

        wo_full = W_out[:, HPC * DK * c: HPC * DK * (c + 1)].T   # [256, D]
        wo_pm = np.ascontiguousarray(
            wo_full.reshape(2, 128, D).transpose(1, 0, 2)).astype(BF_NP)

        in_maps.append({
            "xpm": xpm, "wqa": wqa, "wqb": wqb, "wv": wv_pm, "wo": wo_pm,
            "bqk": bqk_np, "bv": bv_np, "cos4": cos4, "sin4": sin4,
            "maskT": maskT, "idm": idm,
        })

    trace = os.environ.get("KERNEL_TRACE", "0") == "1"
    res = run_bass_kernel_spmd(nc, in_maps, core_ids=list(range(NCORES)),
                               trace=trace)
    if res.exec_time_ns is not None:
        print(f"HW exec time: {res.exec_time_ns} ns")
        if res.instructions_and_trace is not None:
            print(f"trace: {res.instructions_and_trace[1]}")

    acc = np.zeros((B * S, D), dtype=np.float32)
    for c in range(NCORES):
        acc += res.results[c]["pout"].astype(np.float32).reshape(B * S, D)
    out = acc + b_out[None, :]
    return out.reshape(B, S, D)
